# revision 20
# baseline (speedup 1.0000x reference)
"""Trainium2 Bass kernel for nn_BitSwapWrapper.

Reference computation:
    g    = x[rows, idx]                       # one gathered element per row
    u    = coeff * (bitflip(g, bit_pos) - g)
    pert = scatter(zeros_like(x), (rows, idx), u)
    out  = (x + pert) @ W + b

Because pert has exactly one nonzero per row, (x + pert) @ W decomposes as
    out[i, :] = (x @ W)[i, :] + u[i] * W[idx[i], :] + b
so no [B, F] scatter tensor is ever materialized: the kernel streams x
through a K-accumulated matmul and applies the rank-per-row correction with
an indirect-DMA gather of the needed W rows.

Distribution: data-parallel over the batch dim across 8 NeuronCores
(x/idx/bit_positions sharded on dim 0, W/b/coeff replicated), per the
sharding hint. Each core computes its [512, 256] slice of the output.

Matmul dtype is float32r (TensorE reduced-precision fp32 mode): full-rate
on the PE (1 cycle/row at N=256) with ~1e-4 relative error — well inside
the fp32-scale gate — while x streams as raw fp32 bits, which also lets the
bit-flip gather read exact fp32 values straight from the streamed tensor.
"""

import numpy as np

import concourse.bass as bass
import concourse.mybir as mybir
from concourse.bass_utils import run_bass_kernel_spmd
from concourse.tile import TileContext

N_CORES = 8
B, F, O = 4096, 16384, 256
BC = B // N_CORES        # 512 batch rows per core
P = 128
KC = F // P              # 128 contraction chunks
MB = BC // P             # 4 output row-blocks per core

F32 = mybir.dt.float32
F32R = mybir.dt.float32r
I32 = mybir.dt.int32


def _split_multi_waits(nc):
    """This container's walrus build rejects more than one sync-wait command
    per instruction; split extras onto single-wait NOPs on the same engine."""
    cur_bb = nc.cur_bb.bb
    for f in nc.m.functions:
        for bb in f.blocks:
            il = bb.instructions
            i = 0
            while i < len(il):
                ins = il[i]
                si = getattr(ins, "sync_info", None)
                if si is not None and si.on_wait and len(si.on_wait) > 1:
                    waits = list(si.on_wait)
                    extra, keep = waits[:-1], waits[-1:]
                    carriers = []
                    for w in extra:
                        nop = nc.engines[ins.engine].nop(nofuse=True).ins
                        tail = cur_bb.instructions.pop()
                        assert tail is nop
                        nop.sync_info = mybir.SyncInfo(on_wait=[w], on_update=[])
                        carriers.append(nop)
                    ins.sync_info = mybir.SyncInfo(
                        on_wait=keep, on_update=list(si.on_update or [])
                    )
                    il[i:i] = carriers
                    i += len(carriers)
                i += 1


def build(reps=1, stream_bufs=12, cpg=2, mm_bf16=False, with_bias=True, ws_act_ring=False, wstat=False):
    MMDT = mybir.dt.bfloat16 if mm_bf16 else F32R
    nc = bass.Bass("TRN2", target_bir_lowering=False, debug=False)
    xt = nc.dram_tensor("xt", [F, BC], MMDT, kind="ExternalInput").ap()
    w = nc.dram_tensor("w", [F, O], MMDT, kind="ExternalInput").ap()
    wf = (nc.dram_tensor("wf", [F, O], F32, kind="ExternalInput").ap()
          if mm_bf16 else None)
    gh = (nc.dram_tensor("gh", [BC], F32, kind="ExternalInput").ap()
          if mm_bf16 else None)
    bb_ = nc.dram_tensor("b", [O], MMDT, kind="ExternalInput").ap()
    coeff = nc.dram_tensor("coeff", [P, 1], F32, kind="ExternalInput").ap()
    idx = nc.dram_tensor("idx", [BC], I32, kind="ExternalInput").ap()
    bpos = nc.dram_tensor("bpos", [BC], I32, kind="ExternalInput").ap()
    out = nc.dram_tensor("out", [O, BC] if wstat else [BC, O], F32,
                         kind="ExternalOutput").ap()

    # fp32 bit views of the f32r-typed streams (same bytes)
    if mm_bf16:
        xt_flat_f32 = None
        w_f32 = wf
    else:
        xt_flat_f32 = xt.bitcast(F32).rearrange("a b -> (a b)")[:, None]
        w_f32 = w.bitcast(F32)

    with TileContext(nc) as tc:
        with (
            tc.tile_pool(name="stream", bufs=stream_bufs) as stream,
            tc.tile_pool(name="consts", bufs=1) as consts,
            tc.tile_pool(name="epi", bufs=1) as epi,
            tc.tile_pool(name="psum", bufs=1, space="PSUM") as psum,
        ):
            ones_i = consts.tile([P, 1], I32, name="ones_i")
            nc.vector.memset(ones_i[:], 1)
            if with_bias:
                ones_f = consts.tile([1, P], F32, name="ones_f")
                nc.vector.memset(ones_f[:], 1.0)
                ones_row = consts.tile([1, P], MMDT, name="ones_row")
                nc.vector.tensor_copy(out=ones_row[:], in_=ones_f[:])
                brow = consts.tile([1, O], MMDT, name="brow")
                nc.sync.dma_start(out=brow[:], in_=bb_[None, :])
            coeff_b = consts.tile([P, 1], F32, name="coeff_b")
            nc.gpsimd.dma_start(out=coeff_b[:], in_=coeff[:])

            for _ in range(reps):
                if wstat:
                    psums = [
                        psum.tile([P, BC], F32, tag=f"pso{h}", name=f"pso{h}")
                        for h in range(O // P)
                    ]
                else:
                    psums = [
                        psum.tile([P, O], F32, tag=f"ps{m}", name=f"ps{m}")
                        for m in range(MB)
                    ]
                corrs = []
                def emit_prep(m):
                    rows = slice(m * P, (m + 1) * P)
                    idxt = epi.tile([P, 1], I32, tag=f"idxt{m}", name=f"idxt{m}")
                    nc.sync.dma_start(out=idxt[:], in_=idx[rows, None])
                    bpt = epi.tile([P, 1], I32, tag=f"bpt{m}", name=f"bpt{m}")
                    nc.sync.dma_start(out=bpt[:], in_=bpos[rows, None])

                    # flat offset of x[i, idx[i]] inside xt[F, BC]: idx*BC + i
                    if mm_bf16:
                        iot = None
                    else:
                        iot = epi.tile([P, 1], I32, tag=f"iot{m}", name=f"iot{m}")
                    if not mm_bf16:
                        nc.gpsimd.iota(
                            iot[:], [[0, 1]], base=m * P, channel_multiplier=1
                        )
                        flat = epi.tile([P, 1], I32, tag=f"flat{m}", name=f"flat{m}")
                        nc.vector.tensor_scalar(
                            flat[:], idxt[:], BC, None, mybir.AluOpType.mult
                        )
                        nc.vector.tensor_tensor(
                            out=flat[:], in0=flat[:], in1=iot[:],
                            op=mybir.AluOpType.add,
                        )
                    g = epi.tile([P, 1], F32, tag=f"g{m}", name=f"g{m}")
                    if mm_bf16:
                        nc.sync.dma_start(out=g[:], in_=gh[rows, None])
                    else:
                        nc.gpsimd.indirect_dma_start(
                            out=g[:], out_offset=None,
                            in_=xt_flat_f32,
                            in_offset=bass.IndirectOffsetOnAxis(ap=flat[:, :1], axis=0),
                        )
                    # u = coeff * (bitflip(g) - g)
                    mask = epi.tile([P, 1], I32, tag=f"mask{m}", name=f"mask{m}")
                    nc.vector.tensor_scalar(
                        mask[:], ones_i[:], bpt[:, :1], None,
                        mybir.AluOpType.logical_shift_left,
                    )
                    gflip = epi.tile([P, 1], I32, tag=f"gflip{m}", name=f"gflip{m}")
                    nc.vector.tensor_tensor(
                        out=gflip[:], in0=g[:].bitcast(I32), in1=mask[:],
                        op=mybir.AluOpType.bitwise_xor,
                    )
                    u = epi.tile([P, 1], F32, tag=f"u{m}", name=f"u{m}")
                    nc.vector.tensor_tensor(
                        out=u[:], in0=gflip[:].bitcast(F32), in1=g[:],
                        op=mybir.AluOpType.subtract,
                    )
                    nc.vector.tensor_tensor(
                        out=u[:], in0=u[:], in1=coeff_b[:],
                        op=mybir.AluOpType.mult,
                    )
                    # gather W[idx[i], :] rows and apply the correction
                    if wstat:
                        wg = epi.tile([P, O], MMDT, tag=f"wg{m}", name=f"wg{m}")
                        nc.gpsimd.indirect_dma_start(
                            out=wg[:], out_offset=None,
                            in_=w[:],
                            in_offset=bass.IndirectOffsetOnAxis(
                                ap=idxt[:, :1], axis=0),
                        )
                        # diag(u): psum'[o,i] += sum_k wg[k,o]*diag[k,i]
                        diag_f = epi.tile([P, P], F32, tag=f"diagf{m}",
                                          name=f"diagf{m}")
                        nc.gpsimd.affine_select(
                            out=diag_f[:],
                            in_=u[:, :1].to_broadcast([P, P]),
                            pattern=[[-1, P]],
                            compare_op=mybir.AluOpType.is_equal,
                            fill=0.0,
                            base=0,
                            channel_multiplier=1,
                        )
                        diag = epi.tile([P, P], MMDT, tag=f"diag{m}",
                                        name=f"diag{m}")
                        nc.vector.tensor_copy(out=diag[:], in_=diag_f[:])
                        corrs.append((wg, diag))
                    else:
                        wg = epi.tile([P, O], F32, tag=f"wg{m}", name=f"wg{m}")
                        nc.gpsimd.indirect_dma_start(
                            out=wg[:], out_offset=None,
                            in_=w_f32[:],
                            in_offset=bass.IndirectOffsetOnAxis(
                                ap=idxt[:, :1], axis=0),
                        )
                        corr = epi.tile([P, O], F32, tag=f"corr{m}",
                                        name=f"corr{m}")
                        nc.vector.tensor_scalar(
                            corr[:], wg[:], u[:, :1], None,
                            mybir.AluOpType.mult
                        )
                        corrs.append(corr)


                CPG = cpg  # k-chunks per DMA slab
                slabs = [(i * CPG, CPG) for i in range(KC // CPG - 1)]
                slabs += [(KC - CPG + j, 1) for j in range(CPG)]
                for k4, (k0, nch) in enumerate(slabs):
                    r0 = k0 * P
                    xs = stream.tile([P, nch * BC], MMDT, tag="xs",
                                     name="xs", padded_shape=[P, CPG * BC])
                    ws = stream.tile([P, nch * O], MMDT, tag="ws",
                                     name="ws", padded_shape=[P, CPG * O])
                    nc.sync.dma_start(
                        out=xs[:].rearrange("p (c b) -> p c b", c=nch),
                        in_=xt[r0:r0 + nch * P, :].rearrange(
                            "(c p) b -> p c b", p=P),
                    )
                    (nc.scalar if ws_act_ring else nc.sync).dma_start(
                        out=ws[:].rearrange("p (c o) -> p c o", c=nch),
                        in_=w[r0:r0 + nch * P, :].rearrange(
                            "(c p) o -> p c o", p=P),
                    )
                    if 1 <= k4 <= MB:
                        # interleave correction prep behind the first slabs:
                        # dependency-free w.r.t. the stream, scheduled at
                        # lower priority so it fills DMA/engine gaps early
                        emit_prep(k4 - 1)
                    for c in range(nch):
                        if wstat:
                            for h in range(O // P):
                                nc.tensor.matmul(
                                    psums[h][:],
                                    lhsT=ws[:, c * O + h * P:c * O + (h + 1) * P],
                                    rhs=xs[:, c * BC:(c + 1) * BC],
                                    start=(k4 == 0 and c == 0),
                                    stop=False,
                                )
                        else:
                            last_slab = k4 == len(slabs) - 1
                            for m in range(MB):
                                nc.tensor.matmul(
                                    psums[m][:],
                                    lhsT=xs[:, c * BC + m * P:c * BC + (m + 1) * P],
                                    rhs=ws[:, c * O:(c + 1) * O],
                                    start=(k4 == 0 and c == 0),
                                    stop=(not with_bias and last_slab
                                          and c == nch - 1),
                                )
                for m in range(len(corrs), MB):
                    emit_prep(m)  # safety for large cpg (few slabs)
                if wstat:
                    assert not with_bias, "wstat path assumes b == 0"
                    # fold the correction into PSUM: one diag(u) matmul per
                    # (m-block, o-half); the last one closes each group
                    for m in range(MB):
                        wg, diag = corrs[m]
                        for h in range(O // P):
                            nc.tensor.matmul(
                                psums[h][:, m * P:(m + 1) * P],
                                lhsT=wg[:, h * P:(h + 1) * P],
                                rhs=diag[:],
                                start=False,
                                stop=(m == MB - 1),
                                skip_group_check=True,
                            )
                    for h in range(O // P):
                        outt = epi.tile([P, BC], F32, tag=f"outth{h}",
                                        name=f"outth{h}")
                        nc.vector.tensor_copy(out=outt[:], in_=psums[h][:])
                        eng = nc.sync if h % 2 == 0 else nc.scalar
                        eng.dma_start(
                            out=out[h * P:(h + 1) * P, :], in_=outt[:])
                else:
                    if with_bias:
                        # bias: psum[m][i,:] += 1*b[:] (K=1 matmul ends group)
                        for m in range(MB):
                            nc.tensor.matmul(
                                psums[m][:],
                                lhsT=ones_row[:],
                                rhs=brow[:],
                                start=False,
                                stop=True,
                            )
                    for m in range(MB):
                        rows = slice(m * P, (m + 1) * P)
                        outt = epi.tile([P, O], F32, tag=f"outt{m}",
                                        name=f"outt{m}")
                        nc.vector.tensor_tensor(
                            out=outt[:], in0=psums[m][:], in1=corrs[m][:],
                            op=mybir.AluOpType.add,
                        )
                        eng = nc.sync if m % 2 == 0 else nc.scalar
                        eng.dma_start(out=out[rows, :], in_=outt[:])

    _split_multi_waits(nc)
    return nc


_NC_CACHE = {}


def _get_nc(reps=1, with_bias=True):
    key = (reps, with_bias)
    if key not in _NC_CACHE:
        _NC_CACHE[key] = build(reps, with_bias=with_bias)
    return _NC_CACHE[key]


def make_in_maps(x, W, b, bitswap_coeff, idx, bit_positions, mm_bf16=False):
    x = np.asarray(x, dtype=np.float32)
    Wf = np.ascontiguousarray(W, dtype=np.float32)
    b = np.ascontiguousarray(b, dtype=np.float32)
    coeff = np.full((128, 1), np.asarray(bitswap_coeff, dtype=np.float32))
    idx = np.asarray(idx, dtype=np.int32)
    if mm_bf16:
        import ml_dtypes
        xT = x.astype(ml_dtypes.bfloat16).T
        Wmm = Wf.astype(ml_dtypes.bfloat16)
        bmm = b.astype(ml_dtypes.bfloat16)
        g_all = x[np.arange(B), idx].astype(np.float32)
    else:
        xT = x.T  # [F, B] view; per-core slices stay views until concat
        Wmm, bmm, g_all = Wf, b, None
    in_maps = []
    for c in range(N_CORES):
        cols = slice(c * BC, (c + 1) * BC)
        m = {
            "xt": xT[:, cols],
            "w": Wmm,
            "b": bmm,
            "coeff": coeff,
            "idx": np.ascontiguousarray(idx[cols]),
            "bpos": np.ascontiguousarray(bit_positions[cols], dtype=np.int32),
        }
        if mm_bf16:
            m["wf"] = Wf
            m["gh"] = np.ascontiguousarray(g_all[cols])
        in_maps.append(m)
    return in_maps


def kernel(x, W, b, bitswap_coeff, idx, bit_positions):
    with_bias = bool(np.any(np.asarray(b)))
    nc = _get_nc(with_bias=with_bias)
    in_maps = make_in_maps(x, W, b, bitswap_coeff, idx, bit_positions)
    res = run_bass_kernel_spmd(nc, in_maps, core_ids=list(range(N_CORES)))
    return np.concatenate([res.results[c]["out"] for c in range(N_CORES)], axis=0)

